# revision 5
# baseline (speedup 1.0000x reference)
"""Trainium2 Bass kernel for cross "efficient attention".

Reference computation (per batch b, head h, with C=128, HEADS=8, hc=16, n=16384):
    k = x2[b].reshape(HEADS, hc, n); v = x1[b].reshape(HEADS, hc, n)
    key_sm   = softmax(k, axis=-1)          # over n
    query_sm = softmax(k, axis=1)           # over hc (head channels)
    context  = key_sm @ v^T                 # (hc, hc)
    out[b,h] = context^T @ query_sm         # (hc, n)

Sharding: data-parallel over batch B=8 across the 8 NeuronCores (no
collectives).  Inputs are ~N(0,1) so softmax needs no max-subtraction.

Layout strategy: the host pre-transposes BOTH inputs into an
"A-layout" [128, N] bf16 array (partition p holds n = j*128+p for all
channel-blocks j), so every DMA is a plain contiguous slice per
partition.  x1 additionally gets a ones-column appended per block
(129 channels) so the context matmul emits rowsums for free.

On-device dataflow per slab of n:
    eT   = exp(x2T)                                  (scalar)
    csT  = per-head colsums: segmented 16-reduce     (vector, bf16 2x)
    rcp  = 1/csT duplicated into packed pairs        (vector)
    qsmT = eT * bcast(rcp)   == query_sm transposed  (vector, 2x via pairs)
    qsm  = PE-transpose(qsmT) -> natural [C, n]      (PE + scalar/pool copies)
    ctx += eT_j^T @ [vT_j | 1] per 128-block         (PE, accumulated in PSUM)
Then bd = blockdiag(ctx * 1/rowsum), and the tail streams
    out[:, t] = bd^T @ qsm[:, t]                     (PE + cast copies + DMA)
with no tail elementwise math.
"""

import numpy as np
from contextlib import ExitStack

B, C, H, W = 8, 128, 128, 128
N = H * W                 # 16384
J = N // 128              # 128 channel-blocks
HEADS, HC = 8, 16
NCORES = 8

# Slab widths: wide while DMA-bound, tapered at the end so the final
# slab's chain (exp->segred->recip->qsm->transpose) is short.
SLABS = [2048] * 7 + [1024, 512, 512]
assert sum(SLABS) == N
GRP = 1024                # transpose-copy group (one PSUM bank of bf16)
QS = 512                  # tail matmul moving-operand tile (one PSUM bank f32)
OT = 2048                 # tail output tile width
NOT = N // OT             # 8

_cache: dict = {}


def _build():
    import concourse.bass as bass
    import concourse.tile as tile
    from concourse import bacc, mybir

    FP32 = mybir.dt.float32
    BF16 = mybir.dt.bfloat16
    AF = mybir.ActivationFunctionType

    nc = bacc.Bacc("TRN2", target_bir_lowering=False, debug=False)

    x2p = nc.dram_tensor("x2p", [128, N], BF16, kind="ExternalInput")
    x1p = nc.dram_tensor("x1p", [128, J * 129], BF16, kind="ExternalInput")
    bd8_in = nc.dram_tensor("bd8", [C, C], BF16, kind="ExternalInput")
    ident_in = nc.dram_tensor("ident", [C, C], BF16, kind="ExternalInput")
    out = nc.dram_tensor("out", [C, N], BF16, kind="ExternalOutput")

    with tile.TileContext(nc) as tc:
        with ExitStack() as ctx:
            persist = ctx.enter_context(tc.tile_pool(name="persist", bufs=1))
            x2ld = ctx.enter_context(tc.tile_pool(name="x2ld", bufs=3))
            vld = ctx.enter_context(tc.tile_pool(name="vld", bufs=3))
            eTp = ctx.enter_context(tc.tile_pool(name="eTp", bufs=3))
            qsp = ctx.enter_context(tc.tile_pool(name="qsp", bufs=3))
            nrm = ctx.enter_context(tc.tile_pool(name="nrm", bufs=2))
            outp = ctx.enter_context(tc.tile_pool(name="outp", bufs=3))
            smalls = ctx.enter_context(tc.tile_pool(name="smalls", bufs=1))

            qsm_nat = persist.tile([C, N], BF16, tag="qsm_nat")
            bd8 = smalls.tile([C, C], BF16, tag="bd8")
            ident = smalls.tile([C, C], BF16, tag="ident")

            with tc.tile_pool(name="pstr", bufs=4, space="PSUM") as ps_tr, \
                 tc.tile_pool(name="psctx", bufs=1, space="PSUM") as ps_ctx:
                ctx_ps = ps_ctx.tile([C, 129], FP32, tag="ctx")

                n_blk = J
                mm_idx = 0
                pending = None   # (eT, vT, n_chunks) of the previous slab

                def emit_ctx(eT, vT, nch):
                    nonlocal mm_idx
                    for j in range(nch):
                        nc.tensor.matmul(
                            ctx_ps[:],
                            eT[:, bass.ts(j, 128)],          # (n0=128, c_k=128)
                            vT[:, j * 129:(j + 1) * 129],    # (n0=128, 129)
                            start=(mm_idx == 0),
                            stop=(mm_idx == n_blk - 1),
                        )
                        mm_idx += 1

                off = 0
                for i, SW in enumerate(SLABS):
                    Ji = SW // 128
                    x2t = x2ld.tile([128, SW], BF16, tag="x2t")
                    nc.sync.dma_start(out=x2t[:], in_=x2p[:, bass.ds(off, SW)])
                    vT = vld.tile([128, Ji * 129], BF16, tag="vT")
                    nc.sync.dma_start(
                        out=vT[:],
                        in_=x1p[:, bass.ds((off // 128) * 129, Ji * 129)],
                    )
                    if i == 0:
                        # constants go down the scalar HWDGE queue in parallel
                        nc.scalar.dma_start(out=ident[:], in_=ident_in[:])
                        nc.scalar.dma_start(out=bd8[:], in_=bd8_in[:])

                    eT = eTp.tile([128, SW], BF16, tag="eT")
                    nc.scalar.activation(eT[:], x2t[:], AF.Exp)

                    # per-head colsums (transposed layout): reduce groups of 16
                    G = Ji * 8
                    csb = nrm.tile([128, G], BF16, tag="csb")
                    with nc.allow_low_precision("bf16 colsum of 16 positives; tol 2e-2"):
                        nc.vector.tensor_reduce(
                            csb[:].rearrange("p (g o) -> p g o", o=1),
                            eT[:].rearrange("p (g c) -> p g c", c=16),
                            mybir.AxisListType.X, mybir.AluOpType.add,
                        )
                    csf = nrm.tile([128, G], FP32, tag="csf")
                    nc.gpsimd.tensor_copy(csf[:], csb[:])
                    rcf = nrm.tile([128, G], FP32, tag="rcf")
                    nc.vector.reciprocal_approx_fast(out=rcf[:], in_=csf[:])
                    # duplicate each reciprocal into a packed bf16 pair so the
                    # qsm multiply keeps an innermost unit-stride (DVE 2x mode)
                    rc2 = nrm.tile([128, G * 2], BF16, tag="rc2")
                    nc.gpsimd.tensor_copy(
                        rc2[:].rearrange("p (g t) -> p g t", t=2),
                        rcf[:, :, None].broadcast_to([128, G, 2]),
                    )
                    qsmT = qsp.tile([128, SW], BF16, tag="qsmT")
                    nc.vector.tensor_mul(
                        qsmT[:].rearrange("p (g a t) -> p g a t", a=8, t=2),
                        eT[:].rearrange("p (g a t) -> p g a t", a=8, t=2),
                        rc2[:].rearrange("p (g t) -> p g t", t=2)[:, :, None, :]
                              .broadcast_to([128, G, 8, 2]),
                    )

                    # ctx matmuls of the PREVIOUS slab first: their inputs are
                    # ready, so the PE never stalls on this slab's qsmT chain.
                    if pending is not None:
                        emit_ctx(*pending)
                    pending = (eT, vT, Ji)

                    # PE transposes qsmT -> natural layout, 128x128 blocks.
                    # PSUM->SBUF copies alternate scalar/vector (gpsimd cannot
                    # read PSUM on TRN2).
                    for g0 in range(0, SW, GRP):
                        gw = min(GRP, SW - g0)
                        te = ps_tr.tile([C, gw], BF16, tag="te")
                        for j in range(gw // 128):
                            nc.tensor.transpose(
                                te[:, bass.ts(j, 128)],
                                qsmT[:, bass.ds(g0 + j * 128, 128)],
                                ident[:],
                            )
                        go = bass.ds(off + g0, gw)
                        if (i + g0 // GRP) % 2 == 0:
                            nc.scalar.copy(qsm_nat[:, go], te[:])
                        else:
                            nc.vector.tensor_copy(qsm_nat[:, go], te[:])
                    off += SW
                emit_ctx(*pending)

                # ---- block-diagonal context weights ----
                rs_rcp = smalls.tile([C, 1], FP32, tag="rs_rcp")
                nc.vector.reciprocal(rs_rcp[:], ctx_ps[:, 128:129])
                scaled = smalls.tile([C, C], BF16, tag="scaled")
                nc.vector.tensor_scalar(
                    scaled[:], ctx_ps[:, 0:128], rs_rcp[:, 0:1], None,
                    mybir.AluOpType.mult,
                )
                bd = smalls.tile([C, C], BF16, tag="bd")
                nc.vector.tensor_mul(bd[:], scaled[:], bd8[:])

            # ---- Tail: attended matmuls, cast, store ----
            with tc.tile_pool(name="psatt", bufs=2, space="PSUM") as ps_att:
                for t in range(NOT):
                    att = ps_att.tile([C, OT], FP32, tag="att")
                    for q in range(OT // QS):
                        nc.tensor.matmul(
                            att[:, bass.ts(q, QS)], bd[:],
                            qsm_nat[:, bass.ds(t * OT + q * QS, QS)],
                        )
                    ot = outp.tile([C, OT], BF16, tag="ot")
                    if t % 8 in (1, 4, 6):
                        nc.vector.tensor_copy(ot[:], att[:])
                    else:
                        nc.scalar.copy(ot[:], att[:])
                    nc.sync.dma_start(out=out[:, bass.ts(t, OT)], in_=ot[:])

    nc.compile()
    return nc


def _get_nc():
    if "nc" not in _cache:
        _cache["nc"] = _build()
    return _cache["nc"]


def _bd8_np() -> np.ndarray:
    import ml_dtypes

    m = np.zeros((C, C), dtype=np.float32)
    for h in range(HEADS):
        m[h * HC:(h + 1) * HC, h * HC:(h + 1) * HC] = 1.0
    return m.astype(ml_dtypes.bfloat16)


def _ident_np() -> np.ndarray:
    import ml_dtypes

    return np.eye(C, dtype=np.float32).astype(ml_dtypes.bfloat16)


def _to_np(a) -> np.ndarray:
    """Materialize to float32 numpy; retry once on a transient bad fetch
    (device-backed arrays have been observed to materialize NaNs once)."""
    out = np.asarray(a, dtype=np.float32)
    if np.isnan(out).any():
        out = np.asarray(a, dtype=np.float32)
    return out


def _in_maps(x1: np.ndarray, x2: np.ndarray) -> list[dict]:
    """Host-side sharding + layout: per-core A-layout bf16 arrays."""
    import ml_dtypes

    BF = ml_dtypes.bfloat16
    x1 = _to_np(x1).reshape(B, C, N)
    x2 = _to_np(x2).reshape(B, C, N)
    # A-layout: arr[b, p, j, c] = x[b, c, j*128 + p]
    x2a = np.ascontiguousarray(
        x2.reshape(B, C, J, 128).transpose(0, 3, 2, 1)).astype(BF)
    x1a = x1.reshape(B, C, J, 128).transpose(0, 3, 2, 1).astype(BF)
    x1e = np.ones((B, 128, J, 129), dtype=BF)
    x1e[..., :128] = x1a
    x2a = x2a.reshape(B, 128, N)
    x1e = x1e.reshape(B, 128, J * 129)
    bd8 = _bd8_np()
    ident = _ident_np()
    return [
        {"x2p": x2a[i], "x1p": x1e[i], "bd8": bd8, "ident": ident}
        for i in range(NCORES)
    ]


def kernel(x1: np.ndarray, x2: np.ndarray) -> np.ndarray:
    from concourse.bass_utils import run_bass_kernel_spmd

    nc = _get_nc()
    in_maps = _in_maps(x1, x2)
    res = run_bass_kernel_spmd(nc, in_maps, core_ids=list(range(NCORES)))
    outs = [np.asarray(res.results[i]["out"], dtype=np.float32) for i in range(NCORES)]
    return np.stack(outs, axis=0).reshape(B, C, H, W)


# revision 9
# speedup vs baseline: 1.0993x; 1.0993x over previous
"""Trainium2 Bass kernel for cross "efficient attention".

Reference computation (per batch b, head h, with C=128, HEADS=8, hc=16, n=16384):
    k = x2[b].reshape(HEADS, hc, n); v = x1[b].reshape(HEADS, hc, n)
    key_sm   = softmax(k, axis=-1)          # over n
    query_sm = softmax(k, axis=1)           # over hc (head channels)
    context  = key_sm @ v^T                 # (hc, hc)
    out[b,h] = context^T @ query_sm         # (hc, n)

Sharding: data-parallel over batch B=8 across the 8 NeuronCores (no
collectives).  Inputs are ~N(0,1) so softmax needs no max-subtraction.

Layout strategy: the host pre-transposes BOTH inputs into an
"A-layout" [128, N] bf16 array (partition p holds n = j*128+p for all
channel-blocks j), so every DMA is a plain contiguous slice per
partition.  x1 additionally gets a ones-column appended per block
(129 channels) so the context matmul emits rowsums for free.

On-device dataflow per slab of n:
    eT   = exp(x2T)                                  (scalar)
    csT  = per-head colsums: segmented 16-reduce     (vector, bf16 2x)
    rcp  = 1/csT duplicated into packed pairs        (vector)
    qsmT = eT * bcast(rcp)   == query_sm transposed  (vector, 2x via pairs)
    qsm  = PE-transpose(qsmT) -> natural [C, n]      (PE + scalar/pool copies)
    ctx += eT_j^T @ [vT_j | 1] per 128-block         (PE, accumulated in PSUM)
Then bd = blockdiag(ctx * 1/rowsum), and the tail streams
    out[:, t] = bd^T @ qsm[:, t]                     (PE + cast copies + DMA)
with no tail elementwise math.
"""

import numpy as np
from contextlib import ExitStack

B, C, H, W = 8, 128, 128, 128
N = H * W                 # 16384
J = N // 128              # 128 channel-blocks
HEADS, HC = 8, 16
NCORES = 8

# Slab widths: wide while DMA-bound, tapered at the end so the final
# slab's chain (exp->segred->recip->qsm->transpose) is short.
SLABS = [2048] * 7 + [1024, 512, 512]
assert sum(SLABS) == N
GRP = 1024                # transpose-copy group (one PSUM bank of bf16)
QS = 512                  # tail matmul moving-operand tile (one PSUM bank f32)
OT = 2048                 # tail output tile width
NOT = N // OT             # 8

_cache: dict = {}


def _build():
    import concourse.bass as bass
    import concourse.tile as tile
    from concourse import bacc, mybir

    FP32 = mybir.dt.float32
    BF16 = mybir.dt.bfloat16
    AF = mybir.ActivationFunctionType

    nc = bacc.Bacc("TRN2", target_bir_lowering=False, debug=False)

    x2p = nc.dram_tensor("x2p", [128, N], BF16, kind="ExternalInput")
    x1p = nc.dram_tensor("x1p", [128, J * 129], BF16, kind="ExternalInput")
    bd8_in = nc.dram_tensor("bd8", [C, C], BF16, kind="ExternalInput")
    ident_in = nc.dram_tensor("ident", [C, C], BF16, kind="ExternalInput")
    out = nc.dram_tensor("out", [C, N], BF16, kind="ExternalOutput")

    with tile.TileContext(nc) as tc:
        with ExitStack() as ctx:
            persist = ctx.enter_context(tc.tile_pool(name="persist", bufs=1))
            x2ld = ctx.enter_context(tc.tile_pool(name="x2ld", bufs=3))
            vld = ctx.enter_context(tc.tile_pool(name="vld", bufs=3))
            eTp = ctx.enter_context(tc.tile_pool(name="eTp", bufs=3))
            qsp = ctx.enter_context(tc.tile_pool(name="qsp", bufs=3))
            nrm = ctx.enter_context(tc.tile_pool(name="nrm", bufs=2))
            outp = ctx.enter_context(tc.tile_pool(name="outp", bufs=3))
            smalls = ctx.enter_context(tc.tile_pool(name="smalls", bufs=1))

            qsm_nat = persist.tile([C, N], BF16, tag="qsm_nat")
            bd8 = smalls.tile([C, C], BF16, tag="bd8")
            ident = smalls.tile([C, C], BF16, tag="ident")

            with tc.tile_pool(name="pstr", bufs=3, space="PSUM") as ps_tr, \
                 tc.tile_pool(name="psctx", bufs=1, space="PSUM") as ps_ctx:
                ctx_ps = ps_ctx.tile([C, 129], FP32, tag="ctx")

                n_blk = J
                mm_idx = 0
                pending = None   # (eT, vT, n_chunks) of the previous slab

                def emit_ctx(eT, vT, nch):
                    nonlocal mm_idx
                    for j in range(nch):
                        nc.tensor.matmul(
                            ctx_ps[:],
                            eT[:, bass.ts(j, 128)],          # (n0=128, c_k=128)
                            vT[:, j * 129:(j + 1) * 129],    # (n0=128, 129)
                            start=(mm_idx == 0),
                            stop=(mm_idx == n_blk - 1),
                        )
                        mm_idx += 1

                off = 0
                for i, SW in enumerate(SLABS):
                    Ji = SW // 128
                    x2t = x2ld.tile([128, SW], BF16, tag="x2t")
                    nc.sync.dma_start(out=x2t[:], in_=x2p[:, bass.ds(off, SW)])
                    vT = vld.tile([128, Ji * 129], BF16, tag="vT")
                    nc.sync.dma_start(
                        out=vT[:],
                        in_=x1p[:, bass.ds((off // 128) * 129, Ji * 129)],
                    )
                    if i == 0:
                        # constants go down the scalar HWDGE queue in parallel
                        nc.scalar.dma_start(out=ident[:], in_=ident_in[:])
                        nc.scalar.dma_start(out=bd8[:], in_=bd8_in[:])

                    eT = eTp.tile([128, SW], BF16, tag="eT")
                    nc.scalar.activation(eT[:], x2t[:], AF.Exp)

                    # per-head colsums (transposed layout): reduce groups of 16
                    G = Ji * 8
                    csb = nrm.tile([128, G], BF16, tag="csb")
                    with nc.allow_low_precision("bf16 colsum of 16 positives; tol 2e-2"):
                        nc.vector.tensor_reduce(
                            csb[:],
                            eT[:].rearrange("p (g c) -> p g c", c=16),
                            mybir.AxisListType.X, mybir.AluOpType.add,
                        )
                    csf = nrm.tile([128, G], FP32, tag="csf")
                    nc.gpsimd.tensor_copy(csf[:], csb[:])
                    rcf = nrm.tile([128, G], FP32, tag="rcf")
                    nc.vector.reciprocal_approx_fast(out=rcf[:], in_=csf[:])
                    # duplicate each reciprocal into a packed bf16 pair so the
                    # qsm multiply keeps an innermost unit-stride (DVE 2x mode)
                    rc2 = nrm.tile([128, G * 2], BF16, tag="rc2")
                    nc.gpsimd.tensor_copy(
                        rc2[:].rearrange("p (g t) -> p g t", t=2),
                        rcf[:, :, None].broadcast_to([128, G, 2]),
                    )
                    qsmT = qsp.tile([128, SW], BF16, tag="qsmT")
                    nc.vector.tensor_mul(
                        qsmT[:].rearrange("p (g a t) -> p g a t", a=8, t=2),
                        eT[:].rearrange("p (g a t) -> p g a t", a=8, t=2),
                        rc2[:].rearrange("p (g t) -> p g t", t=2)[:, :, None, :]
                              .broadcast_to([128, G, 8, 2]),
                    )

                    # ctx matmuls of the PREVIOUS slab first: their inputs are
                    # ready, so the PE never stalls on this slab's qsmT chain.
                    if pending is not None:
                        emit_ctx(*pending)
                    pending = (eT, vT, Ji)

                    # PE transposes qsmT -> natural layout, 128x128 blocks.
                    # One PSUM->SBUF copy per slab, alternating scalar/vector
                    # (gpsimd cannot read PSUM on TRN2).
                    te = ps_tr.tile([C, SW], BF16, tag="te")
                    for j in range(SW // 128):
                        nc.tensor.transpose(
                            te[:, bass.ts(j, 128)],
                            qsmT[:, bass.ds(j * 128, 128)],
                            ident[:],
                        )
                    go = bass.ds(off, SW)
                    if i % 2 == 0:
                        nc.scalar.copy(qsm_nat[:, go], te[:])
                    else:
                        nc.vector.tensor_copy(qsm_nat[:, go], te[:])
                    off += SW
                emit_ctx(*pending)

                # ---- block-diagonal context weights ----
                rs_rcp = smalls.tile([C, 1], FP32, tag="rs_rcp")
                nc.vector.reciprocal(rs_rcp[:], ctx_ps[:, 128:129])
                scaled = smalls.tile([C, C], BF16, tag="scaled")
                nc.vector.tensor_scalar(
                    scaled[:], ctx_ps[:, 0:128], rs_rcp[:, 0:1], None,
                    mybir.AluOpType.mult,
                )
                bd = smalls.tile([C, C], BF16, tag="bd")
                nc.vector.tensor_mul(bd[:], scaled[:], bd8[:])

            # ---- Tail: attended matmuls, cast, store ----
            with tc.tile_pool(name="psatt", bufs=2, space="PSUM") as ps_att:
                for t in range(NOT):
                    att = ps_att.tile([C, OT], FP32, tag="att")
                    for q in range(OT // QS):
                        nc.tensor.matmul(
                            att[:, bass.ts(q, QS)], bd[:],
                            qsm_nat[:, bass.ds(t * OT + q * QS, QS)],
                        )
                    # split each cast between scalar and vector so the
                    # PSUM->SBUF stage keeps up with the store DMA
                    ot = outp.tile([C, OT], BF16, tag="ot")
                    SPL = 1152   # scalar 1152 @1.2G ~= vector 896 f32 @0.96G
                    nc.scalar.copy(ot[:, 0:SPL], att[:, 0:SPL])
                    nc.vector.tensor_copy(ot[:, SPL:OT], att[:, SPL:OT])
                    nc.sync.dma_start(out=out[:, bass.ts(t, OT)], in_=ot[:])

    nc.compile()
    return nc


def _get_nc():
    if "nc" not in _cache:
        _cache["nc"] = _build()
    return _cache["nc"]


def _bd8_np() -> np.ndarray:
    import ml_dtypes

    m = np.zeros((C, C), dtype=np.float32)
    for h in range(HEADS):
        m[h * HC:(h + 1) * HC, h * HC:(h + 1) * HC] = 1.0
    return m.astype(ml_dtypes.bfloat16)


def _ident_np() -> np.ndarray:
    import ml_dtypes

    return np.eye(C, dtype=np.float32).astype(ml_dtypes.bfloat16)


def _to_np(a) -> np.ndarray:
    """Materialize to float32 numpy; retry once on a transient bad fetch
    (device-backed arrays have been observed to materialize NaNs once)."""
    out = np.asarray(a, dtype=np.float32)
    if np.isnan(out).any():
        out = np.asarray(a, dtype=np.float32)
    return out


def _in_maps(x1: np.ndarray, x2: np.ndarray) -> list[dict]:
    """Host-side sharding + layout: per-core A-layout bf16 arrays."""
    import ml_dtypes

    BF = ml_dtypes.bfloat16
    x1 = _to_np(x1).reshape(B, C, N)
    x2 = _to_np(x2).reshape(B, C, N)
    # A-layout: arr[b, p, j, c] = x[b, c, j*128 + p]
    x2a = np.ascontiguousarray(
        x2.reshape(B, C, J, 128).transpose(0, 3, 2, 1)).astype(BF)
    x1a = x1.reshape(B, C, J, 128).transpose(0, 3, 2, 1).astype(BF)
    x1e = np.ones((B, 128, J, 129), dtype=BF)
    x1e[..., :128] = x1a
    x2a = x2a.reshape(B, 128, N)
    x1e = x1e.reshape(B, 128, J * 129)
    bd8 = _bd8_np()
    ident = _ident_np()
    return [
        {"x2p": x2a[i], "x1p": x1e[i], "bd8": bd8, "ident": ident}
        for i in range(NCORES)
    ]


def kernel(x1: np.ndarray, x2: np.ndarray) -> np.ndarray:
    from concourse.bass_utils import run_bass_kernel_spmd

    nc = _get_nc()
    in_maps = _in_maps(x1, x2)
    res = run_bass_kernel_spmd(nc, in_maps, core_ids=list(range(NCORES)))
    outs = [np.asarray(res.results[i]["out"], dtype=np.float32) for i in range(NCORES)]
    return np.stack(outs, axis=0).reshape(B, C, H, W)
